# revision 5
# baseline (speedup 1.0000x reference)
"""VQ codebook cosine-similarity softmax kernel for Trainium2 (8 NeuronCores).

Computes softmax(cos_sim(batch, centroids)) for batch [131072, 1024] f32 and
centroids [256, 1024] f32, data-parallel over the batch dim across 8 cores.

Per-core pipeline (16384 rows), v2 (fp8 + XBAR DMA transpose):
  - SWDGE cast-DMA loads x tiles HBM f32 -> SBUF fp8e4 (matmul operand) in
    XB-tile batches
  - row norms^2 from the fp8 tiles, split column-wise between DVE
    (scalar_tensor_tensor) and ACT (Square), both with accum_out
  - batched rsqrt (0x5f3759df bit trick + 3 Newton steps on DVE); the last
    Newton step folds in 1/CS (centroid prescale) via scalar_tensor_tensor
  - XBAR DMA transpose (dma_start_transpose on the sync HWDGE queue) of each
    [128,1024] fp8 tile viewed as uint16 pairs -> xt8 [128d, 4 blocks, 256]
    where each 128-partition block holds d-pairs (2p, 2p+1): exactly the
    DoubleRow fp8 matmul weights layout.  No PE transposes, no PSUM->SBUF
    copyback.
  - 4 DoubleRow fp8 matmuls per row tile (each contracts 256 d) against the
    pair-interleaved centroid table cn8T -> PSUM f32 [128, 256]
  - softmax: logits = cos in [-1,1] so no max-subtraction needed; ACT
    Exp(scale=1/(||x||*CS) per row) with accum_out denominator; batched DVE
    reciprocal per group; DVE tensor_scalar_mul normalize -> fp16 out
  - fp16 output store (host upcasts to f32); rel err ~7e-3 vs 2e-2 budget
"""

import os
import sys

if "/opt/trn_rl_repo" not in sys.path:
    sys.path.insert(0, "/opt/trn_rl_repo")

import numpy as np

# fp8 DoubleRow is dead on HW: the XBAR u16-pair transpose glues the d-pair
# at stride 1B, but dual-fp8 LDWEIGHTS requires the pair dim at stride%16==0
# (s3_lw_dual_fp8_restrictions) -> fp16 everywhere.
XDT = os.environ.get("KM_XDT", "fp16")  # fp8 | fp16
# columns of each row-tile squared+accumulated on DVE (sts); rest on ACT
SQ_SPLIT = int(os.environ.get("KM_SQ_SPLIT", "640"))
DEN_MODE = os.environ.get("KM_DEN_MODE", "act")  # act (Exp accum) | dve (reduce)
X_BUFS = int(os.environ.get("KM_X_BUFS", "8"))
XT_BUFS = int(os.environ.get("KM_XT_BUFS", "6"))
E_BUFS = int(os.environ.get("KM_E_BUFS", "4"))
PM_BUFS = int(os.environ.get("KM_PM_BUFS", "3"))
SPS_BUFS = int(os.environ.get("KM_SPS_BUFS", "6"))

N, D, K = 131072, 1024, 256
NCORES = 8
NPC = N // NCORES  # rows per core
P = 128  # partitions / tile rows
XB = 4  # row-tiles per load/store DMA batch
G = 16  # row-tiles per norm group (batched rsqrt + reciprocal)
CS = 32.0  # centroid prescale: keeps cn out of fp8e4 subnormals

RSQRT_MAGIC = 0x5F3759DF


def build_bass(npc=NPC):
    from contextlib import ExitStack

    import concourse.bacc as bacc
    import concourse.mybir as mybir
    import concourse.tile as tile

    dt = mybir.dt
    AFT = mybir.ActivationFunctionType
    Alu = mybir.AluOpType
    xdt = dt.float8e4 if XDT == "fp8" else dt.float16

    nt = npc // P  # row tiles
    assert npc % (P * XB) == 0
    ngroups = (nt + G - 1) // G

    nc = bacc.Bacc(
        "TRN2", target_bir_lowering=False, debug=False, num_devices=NCORES
    )
    x_d = nc.dram_tensor("x", [npc, D], dt.float32, kind="ExternalInput")
    c_d = nc.dram_tensor("c", [K, D], dt.float32, kind="ExternalInput")
    o_d = nc.dram_tensor("o", [npc, K], dt.float16, kind="ExternalOutput")

    def emit_rsqrt(nc, dst, src, scratch_a, scratch_b, w, fold=1.0):
        """dst[:, :w] = fold/sqrt(src[:, :w]) via bit trick + 3 Newton steps."""
        srci = src.bitcast(dt.int32)
        dsti = dst.bitcast(dt.int32)
        nc.vector.tensor_scalar(dsti, srci, 1, None, Alu.logical_shift_right)
        # magic - x == (x ^ 0xffffffff) + (magic + 1)  (avoids int negate)
        nc.vector.tensor_scalar(dsti, dsti, -1, None, Alu.bitwise_xor)
        nc.vector.tensor_scalar(dsti, dsti, RSQRT_MAGIC + 1, None, Alu.add)
        for it in range(3):
            nc.vector.tensor_tensor(scratch_a, dst, dst, Alu.mult)
            nc.vector.tensor_tensor(scratch_b, scratch_a, src, Alu.mult)
            nc.vector.tensor_scalar(
                scratch_b, scratch_b, -0.5, 1.5, Alu.mult, Alu.add
            )
            if it == 2 and fold != 1.0:
                nc.vector.scalar_tensor_tensor(
                    dst, dst, fold, scratch_b, Alu.mult, Alu.mult
                )
            else:
                nc.vector.tensor_tensor(dst, dst, scratch_b, Alu.mult)

    # fp8 path: d-blocks of 256 contracted per DoubleRow matmul;
    # fp16 path: d-chunks of 128 per plain matmul
    NB = 4 if XDT == "fp8" else 8

    with tile.TileContext(nc) as tc, ExitStack() as ctx:
        const = ctx.enter_context(tc.tile_pool(name="const", bufs=1))
        # transposed (pair-interleaved for fp8) centroid table
        cnT = const.tile([P, NB * (2 * K if XDT == "fp8" else K)], xdt)
        # per-tile squared row norms (partial sums: a=DVE part, b=ACT part)
        n2a = const.tile([P, max(nt, 1)], dt.float32)
        n2b = const.tile([P, max(nt, 1)], dt.float32)
        denscols = const.tile([P, max(nt, 1)], dt.float32)
        rdenscols = const.tile([P, max(nt, 1)], dt.float32)

        # ---- centroid prep (one-time) ----
        with ExitStack() as _cstack:
            cprep = _cstack.enter_context(tc.tile_pool(name="cprep", bufs=2))
            for h in range(K // P):  # 2 halves of the K=256 centroids
                c32 = cprep.tile([P, D], dt.float32, tag="c32")
                nc.sync.dma_start(c32[:], c_d.ap()[P * h : P * (h + 1), :])
                csq = cprep.tile([P, D], dt.float32, tag="csq")
                cn2 = cprep.tile([P, 1], dt.float32, tag="cn2")
                nc.vector.scalar_tensor_tensor(
                    csq[:], c32[:], 1.0, c32[:], Alu.mult, Alu.mult,
                    accum_out=cn2[:],
                )
                crn = cprep.tile([P, 1], dt.float32, tag="crn")
                csa = cprep.tile([P, 1], dt.float32, tag="csa")
                csb = cprep.tile([P, 1], dt.float32, tag="csb")
                emit_rsqrt(nc, crn[:], cn2[:], csa[:], csb[:], 1)
                # cnrows = c * (CS/||c||)  in the matmul dtype
                cnrows = cprep.tile([P, D], xdt, tag="cnrows")
                nc.vector.tensor_scalar(
                    cnrows[:], c32[:], crn[:], CS, Alu.mult, Alu.mult
                )
                # XBAR transpose into cnT half h (u16 view; for fp8 each u16
                # is an adjacent-d pair, giving the DoubleRow interleave)
                dst = (
                    cnT[:]
                    .bitcast(dt.uint16)
                    .rearrange("p (c k) -> p c k", c=NB)[
                        :, :, P * h : P * (h + 1)
                    ]
                )
                nc.sync.dma_start(
                    dst, cnrows[:].bitcast(dt.uint16), transpose=True
                )

        # ---- main loop ----
        x_pool = ctx.enter_context(tc.tile_pool(name="xm", bufs=X_BUFS))
        xt_pool = ctx.enter_context(tc.tile_pool(name="xt", bufs=XT_BUFS))
        sq_pool = ctx.enter_context(tc.tile_pool(name="sq", bufs=2))
        e_pool = ctx.enter_context(tc.tile_pool(name="e", bufs=E_BUFS))
        pm_pool = ctx.enter_context(tc.tile_pool(name="pm", bufs=PM_BUFS))
        nrm_pool = ctx.enter_context(tc.tile_pool(name="nrm", bufs=4))
        sps_pool = ctx.enter_context(
            tc.tile_pool(name="sps", bufs=SPS_BUFS, space="PSUM")
        )

        sd = max(0, min(D, SQ_SPLIT))
        for g in range(ngroups):
            t0 = g * G
            t1 = min(t0 + G, nt)
            gtiles = range(t0, t1)
            gw = t1 - t0
            # 1) cast-loads (XB row-tiles per DMA)
            xmacs = {}
            for tm in range(t0 // XB, (t1 + XB - 1) // XB):
                xm = x_pool.tile([P, XB * D], xdt, tag="xm")
                src = x_d.ap()[P * XB * tm : P * XB * (tm + 1), :].rearrange(
                    "(s p) d -> p s d", s=XB
                )
                nc.gpsimd.dma_start(
                    xm[:].rearrange("p (s d) -> p s d", s=XB), src
                )
                xmacs[tm] = xm
            # 2) row norms^2, split column-wise across DVE (sts) and ACT (Sq)
            for t in gtiles:
                xm = xmacs[t // XB]
                xs = xm[:, D * (t % XB) : D * (t % XB + 1)]
                if sd > 0:
                    sqa = sq_pool.tile([P, D], xdt, tag="sqa")
                    nc.vector.scalar_tensor_tensor(
                        sqa[:, :sd], xs[:, :sd], 1.0, xs[:, :sd],
                        Alu.mult, Alu.mult, accum_out=n2a[:, t : t + 1],
                    )
                if sd < D:
                    sqb = sq_pool.tile([P, D], xdt, tag="sqb")
                    nc.scalar.activation(
                        sqb[:, sd:], xs[:, sd:], AFT.Square,
                        accum_out=n2b[:, t : t + 1],
                    )
            # 3) batched rsqrt for the group's norms; folds the 1/CS for the
            #    Exp scale (logits arrive premultiplied by CS)
            rng = nrm_pool.tile([P, G], dt.float32, tag="rng")
            nsa = nrm_pool.tile([P, G], dt.float32, tag="nsa")
            nsb = nrm_pool.tile([P, G], dt.float32, tag="nsb")
            n2s = nrm_pool.tile([P, G], dt.float32, tag="n2s")
            if sd == 0:
                n2src = n2b[:, t0:t1]
            elif sd == D:
                n2src = n2a[:, t0:t1]
            else:
                nc.vector.tensor_tensor(
                    n2s[:, :gw], n2a[:, t0:t1], n2b[:, t0:t1], Alu.add
                )
                n2src = n2s[:, :gw]
            emit_rsqrt(
                nc, rng[:, :gw], n2src, nsa[:, :gw], nsb[:, :gw], gw,
                fold=1.0 / CS,
            )
            # 4) per row-tile: XBAR transpose -> DoubleRow matmuls -> exp
            for tm in range(t0 // XB, (t1 + XB - 1) // XB):
                bt0 = max(t0, tm * XB)
                bt1 = min(t1, (tm + 1) * XB)
                for t in range(bt0, bt1):
                    xm = xmacs[t // XB]
                    xs = xm[:, D * (t % XB) : D * (t % XB + 1)]
                    xt = xt_pool.tile([P, D], xdt, tag="xt")
                    nbf = (D // 2 if XDT == "fp8" else D) // P  # u16 blocks
                    nc.sync.dma_start(
                        xt[:]
                        .bitcast(dt.uint16)
                        .rearrange("p (c n) -> p c n", c=nbf),
                        xs.bitcast(dt.uint16),
                        transpose=True,
                    )
                    sps = sps_pool.tile([P, K], dt.float32, tag="sps")
                    if XDT == "fp8":
                        for b in range(NB):
                            lhsT = xt[:, 256 * b : 256 * (b + 1)].rearrange(
                                "p (n two) -> p two n", two=2
                            )
                            rhs = cnT[
                                :, 2 * K * b : 2 * K * (b + 1)
                            ].rearrange("p (k two) -> p two k", two=2)
                            nc.tensor.matmul(
                                sps[:], lhsT, rhs,
                                start=(b == 0), stop=(b == NB - 1),
                                perf_mode=mybir.MatmulPerfMode.DoubleRow,
                            )
                    else:
                        for b in range(NB):
                            nc.tensor.matmul(
                                sps[:],
                                xt[:, P * b : P * (b + 1)],
                                cnT[:, K * b : K * (b + 1)],
                                start=(b == 0), stop=(b == NB - 1),
                            )
                    if t == bt0:
                        e = e_pool.tile([P, XB * K], dt.float16, tag="e")
                    j = t - t0
                    nc.scalar.activation(
                        e[:, K * (t % XB) : K * (t % XB + 1)], sps[:],
                        AFT.Exp, scale=rng[:, j : j + 1],
                        accum_out=denscols[:, t : t + 1],
                    )
                nc.vector.reciprocal(
                    rdenscols[:, bt0:bt1], denscols[:, bt0:bt1]
                )
                pm = pm_pool.tile([P, XB * K], dt.float16, tag="pm")
                for t in range(bt0, bt1):
                    nc.vector.tensor_scalar_mul(
                        pm[:, K * (t % XB) : K * (t % XB + 1)],
                        e[:, K * (t % XB) : K * (t % XB + 1)],
                        rdenscols[:, t : t + 1],
                    )
                dst = o_d.ap()[
                    P * XB * tm : P * XB * (tm + 1), :
                ].rearrange("(s p) k -> p s k", s=XB)
                nc.sync.dma_start(
                    dst, pm[:].rearrange("p (s k) -> p s k", s=XB)
                )

    nc.compile()
    return nc


_cache = {}


def _get_nc(npc=NPC):
    if npc not in _cache:
        _cache[npc] = build_bass(npc)
    return _cache[npc]


def kernel(batch: np.ndarray, centroids: np.ndarray) -> np.ndarray:
    from concourse.bass_utils import run_bass_kernel_spmd

    assert batch.shape == (N, D) and centroids.shape == (K, D)
    batch = np.ascontiguousarray(batch, dtype=np.float32)
    centroids = np.ascontiguousarray(centroids, dtype=np.float32)

    nc = _get_nc()
    in_maps = [
        {"x": batch[i * NPC : (i + 1) * NPC], "c": centroids}
        for i in range(NCORES)
    ]
    res = run_bass_kernel_spmd(nc, in_maps, core_ids=list(range(NCORES)))
    out = np.concatenate(
        [np.asarray(res.results[i]["o"]) for i in range(NCORES)], axis=0
    )
    return out.astype(np.float32)


# revision 11
# speedup vs baseline: 1.6245x; 1.6245x over previous
"""VQ codebook cosine-similarity softmax kernel for Trainium2 (8 NeuronCores).

Computes softmax(cos_sim(batch, centroids)) for batch [131072, 1024] f32 and
centroids [256, 1024] f32, data-parallel over the batch dim across 8 cores.

Per-core pipeline (16384 rows), v3 (fp8 DoubleRow + batched u16 copyback):
  - SWDGE cast-DMA loads x tiles HBM f32 -> SBUF fp8e4 (halves SBUF traffic,
    feeds double-pumped fp8 matmuls; rel err ~7e-3 vs the 2e-2 budget)
  - row norms^2 on the fp8 tiles, split column-wise between DVE
    (scalar_tensor_tensor) and ACT (Square), both with accum_out
  - batched rsqrt (0x5f3759df bit trick + 3 Newton steps on DVE); the last
    Newton step folds in 1/CS (centroid prescale) for the Exp scale
  - PE transposes each [128,128] fp8 block into a [128, XB*1024] PSUM strip;
    ONE tensor_copy per XB batch moves it to SBUF through a uint16 bitcast
    (2-byte dtype unlocks the DVE 2x mode; fp8 would run 1x)
  - 4 DoubleRow fp8 matmuls per row tile (contraction 256 each): weights
    xt8[:, (two n)] pair stride 128B, moving cnT[:, (two k)] pair stride
    256B - the block layout dual-fp8 LDWEIGHTS requires (stride%16==0)
  - softmax: logits = cos in [-1,1] so no max-subtraction needed; ACT
    Exp(scale=1/(||x||*CS)) with accum_out denominator; batched DVE
    reciprocal per XB; DVE tensor_scalar_mul (4x mode) normalize -> fp16
  - fp16 output store (host upcasts to f32)
"""

import os
import sys

if "/opt/trn_rl_repo" not in sys.path:
    sys.path.insert(0, "/opt/trn_rl_repo")

import numpy as np

XDT = os.environ.get("KM_XDT", "fp8")  # fp8 | fp16
# columns of each row-tile squared+accumulated on DVE (sts); rest on ACT
SQ_SPLIT = int(os.environ.get("KM_SQ_SPLIT", "768"))
DEN_MODE = os.environ.get("KM_DEN_MODE", "dve")  # act (Exp accum) | dve (reduce)
MUL_MODE = os.environ.get("KM_MUL_MODE", "dve")  # dve (ts_mul) | act (Copy scale)
CB_MODE = os.environ.get("KM_CB_MODE", "dve")  # copyback: dve | split
CB_F1 = int(os.environ.get("KM_CB_F1", "3072"))  # split: u16 cols on DVE
X_BUFS = int(os.environ.get("KM_X_BUFS", "8"))
XT_BUFS = int(os.environ.get("KM_XT_BUFS", "3"))
E_BUFS = int(os.environ.get("KM_E_BUFS", "4"))
PM_BUFS = int(os.environ.get("KM_PM_BUFS", "3"))
SPS_BUFS = int(os.environ.get("KM_SPS_BUFS", "4"))
TPS_BUFS = int(os.environ.get("KM_TPS_BUFS", "2"))

N, D, K = 131072, 1024, 256
NCORES = 8
NPC = N // NCORES  # rows per core
P = 128  # partitions / tile rows
XB = 4  # row-tiles per load/store DMA batch and copyback strip
G = 16  # row-tiles per norm group (batched rsqrt)
CS = 32.0  # centroid prescale: keeps cn out of fp8e4 subnormals

RSQRT_MAGIC = 0x5F3759DF


def build_bass(npc=NPC):
    from contextlib import ExitStack

    import concourse.bacc as bacc
    import concourse.mybir as mybir
    import concourse.tile as tile
    from concourse.masks import make_identity

    dt = mybir.dt
    AFT = mybir.ActivationFunctionType
    Alu = mybir.AluOpType
    xdt = dt.float8e4 if XDT == "fp8" else dt.float16

    nt = npc // P  # row tiles
    assert npc % (P * XB) == 0
    ngroups = (nt + G - 1) // G
    ND = D // P  # 128-wide d-chunks (8)

    nc = bacc.Bacc(
        "TRN2", target_bir_lowering=False, debug=False, num_devices=NCORES
    )
    x_d = nc.dram_tensor("x", [npc, D], dt.float32, kind="ExternalInput")
    c_d = nc.dram_tensor("c", [K, D], dt.float32, kind="ExternalInput")
    o_d = nc.dram_tensor("o", [npc, K], dt.float16, kind="ExternalOutput")

    def emit_rsqrt(nc, dst, src, scratch_a, scratch_b, w, fold=1.0):
        """dst[:, :w] = fold/sqrt(src[:, :w]) via bit trick + 3 Newton steps."""
        srci = src.bitcast(dt.int32)
        dsti = dst.bitcast(dt.int32)
        nc.vector.tensor_scalar(dsti, srci, 1, None, Alu.logical_shift_right)
        nc.vector.tensor_scalar(dsti, dsti, -1, None, Alu.bitwise_xor)
        nc.vector.tensor_scalar(dsti, dsti, RSQRT_MAGIC + 1, None, Alu.add)
        for it in range(3):
            nc.vector.tensor_tensor(scratch_a, dst, dst, Alu.mult)
            nc.vector.tensor_tensor(scratch_b, scratch_a, src, Alu.mult)
            nc.vector.tensor_scalar(
                scratch_b, scratch_b, -0.5, 1.5, Alu.mult, Alu.add
            )
            if it == 2 and fold != 1.0:
                nc.vector.scalar_tensor_tensor(
                    dst, dst, fold, scratch_b, Alu.mult, Alu.mult
                )
            else:
                nc.vector.tensor_tensor(dst, dst, scratch_b, Alu.mult)

    def make_antidiag(nc, ap):
        """Antidiagonal permutation: ap[x, y] = 1.0 iff x + y == P-1."""
        nc.gpsimd.memset(ap, 0.0)
        nc.gpsimd.affine_select(
            out=ap,
            in_=ap,
            compare_op=Alu.not_equal,
            fill=1.0,
            base=-(P - 1),
            pattern=[[1, P]],
            channel_multiplier=1,
        )

    with tile.TileContext(nc) as tc, ExitStack() as ctx:
        const = ctx.enter_context(tc.tile_pool(name="const", bufs=1))
        ident = const.tile([P, P], xdt)
        make_identity(nc, ident[:])
        if XDT == "fp8":
            # fp8 PE transposes must write element-step-2 (pair-interleaved)
            # outputs; x transposes use a flipped identity so the column
            # reversal of DoubleRowSwInterleave weight reads cancels out.
            identa = const.tile([P, P], xdt)
            make_antidiag(nc, identa[:])

        # cnT: [128 d-in-chunk, ND*K] with chunk b at cols [K*b, K*b+K)
        cnT = const.tile([P, ND * K], xdt)
        n2a = const.tile([P, max(nt, 1)], dt.float32)
        n2b = const.tile([P, max(nt, 1)], dt.float32)
        denscols = const.tile([P, max(nt, 1)], dt.float32)
        rdenscols = const.tile([P, max(nt, 1)], dt.float32)

        # ---- centroid prep (one-time) ----
        with ExitStack() as _cstack:
            cprep = _cstack.enter_context(tc.tile_pool(name="cprep", bufs=2))
            cpsum = _cstack.enter_context(
                tc.tile_pool(name="cpsum", bufs=2, space="PSUM")
            )
            for h in range(K // P):  # 2 halves of the K=256 centroids
                c32 = cprep.tile([P, D], dt.float32, tag="c32")
                nc.sync.dma_start(c32[:], c_d.ap()[P * h : P * (h + 1), :])
                csq = cprep.tile([P, D], dt.float32, tag="csq")
                cn2 = cprep.tile([P, 1], dt.float32, tag="cn2")
                nc.vector.scalar_tensor_tensor(
                    csq[:], c32[:], 1.0, c32[:], Alu.mult, Alu.mult,
                    accum_out=cn2[:],
                )
                crn = cprep.tile([P, 1], dt.float32, tag="crn")
                csa = cprep.tile([P, 1], dt.float32, tag="csa")
                csb = cprep.tile([P, 1], dt.float32, tag="csb")
                emit_rsqrt(nc, crn[:], cn2[:], csa[:], csb[:], 1)
                # cnrows = c * (CS/||c||) in the matmul dtype
                cnrows = cprep.tile([P, D], xdt, tag="cnrows")
                nc.vector.tensor_scalar(
                    cnrows[:], c32[:], crn[:], CS, Alu.mult, Alu.mult
                )
                if XDT == "fp8":
                    # chunk pairs (2c, 2c+1) interleaved at element step 2:
                    # cnT[p, 512c + 2k + i] = cn[k-half h, d=128(2c+i)+p]
                    for cpr in range(ND // 2):
                        pt = cpsum.tile([P, 2 * P], xdt, tag="ct_ps")
                        pt2 = pt[:].rearrange("p (n two) -> p two n", two=2)
                        for i in range(2):
                            nc.tensor.transpose(
                                pt2[:, i],
                                cnrows[
                                    :, P * (2 * cpr + i) : P * (2 * cpr + i + 1)
                                ],
                                ident[:],
                            )
                        nc.vector.tensor_copy(
                            cnT[:, 2 * K * cpr + 2 * P * h :
                                2 * K * cpr + 2 * P * (h + 1)].bitcast(
                                dt.uint16
                            ),
                            pt[:].bitcast(dt.uint16),
                        )
                else:
                    for b in range(ND):
                        pt = cpsum.tile([P, P], xdt, tag="ct_ps")
                        nc.tensor.transpose(
                            pt[:], cnrows[:, P * b : P * (b + 1)], ident[:]
                        )
                        nc.vector.tensor_copy(
                            cnT[:, K * b + P * h : K * b + P * h + P], pt[:]
                        )

        # ---- main loop ----
        x_pool = ctx.enter_context(tc.tile_pool(name="xm", bufs=X_BUFS))
        xt_pool = ctx.enter_context(tc.tile_pool(name="xt", bufs=XT_BUFS))
        sq_pool = ctx.enter_context(tc.tile_pool(name="sq", bufs=2))
        e_pool = ctx.enter_context(tc.tile_pool(name="e", bufs=E_BUFS))
        pm_pool = ctx.enter_context(tc.tile_pool(name="pm", bufs=PM_BUFS))
        nrm_pool = ctx.enter_context(tc.tile_pool(name="nrm", bufs=4))
        den_pool = ctx.enter_context(tc.tile_pool(name="den", bufs=4))
        tps_pool = ctx.enter_context(
            tc.tile_pool(name="tps", bufs=TPS_BUFS, space="PSUM")
        )
        sps_pool = ctx.enter_context(
            tc.tile_pool(name="sps", bufs=SPS_BUFS, space="PSUM")
        )

        sd = max(0, min(D, SQ_SPLIT))
        u16_per_tile = D // 2 if XDT == "fp8" else D
        for g in range(ngroups):
            t0 = g * G
            t1 = min(t0 + G, nt)
            gtiles = range(t0, t1)
            gw = t1 - t0
            # 1) cast-loads (XB row-tiles per DMA)
            xmacs = {}
            for tm in range(t0 // XB, (t1 + XB - 1) // XB):
                xm = x_pool.tile([P, XB * D], xdt, tag="xm")
                src = x_d.ap()[P * XB * tm : P * XB * (tm + 1), :].rearrange(
                    "(s p) d -> p s d", s=XB
                )
                nc.gpsimd.dma_start(
                    xm[:].rearrange("p (s d) -> p s d", s=XB), src
                )
                xmacs[tm] = xm
            # 2) row norms^2, split column-wise across DVE (sts) and ACT (Sq)
            for t in gtiles:
                xm = xmacs[t // XB]
                xs = xm[:, D * (t % XB) : D * (t % XB + 1)]
                if sd > 0:
                    sqa = sq_pool.tile([P, D], xdt, tag="sqa")
                    nc.vector.scalar_tensor_tensor(
                        sqa[:, :sd], xs[:, :sd], 1.0, xs[:, :sd],
                        Alu.mult, Alu.mult, accum_out=n2a[:, t : t + 1],
                    )
                if sd < D:
                    sqb = sq_pool.tile([P, D], xdt, tag="sqb")
                    nc.scalar.activation(
                        sqb[:, sd:], xs[:, sd:], AFT.Square,
                        accum_out=n2b[:, t : t + 1],
                    )
            # 3) batched rsqrt; folds 1/CS into the Exp scale
            rng = nrm_pool.tile([P, G], dt.float32, tag="rng")
            nsa = nrm_pool.tile([P, G], dt.float32, tag="nsa")
            nsb = nrm_pool.tile([P, G], dt.float32, tag="nsb")
            n2s = nrm_pool.tile([P, G], dt.float32, tag="n2s")
            if sd == 0:
                n2src = n2b[:, t0:t1]
            elif sd == D:
                n2src = n2a[:, t0:t1]
            else:
                nc.vector.tensor_tensor(
                    n2s[:, :gw], n2a[:, t0:t1], n2b[:, t0:t1], Alu.add
                )
                n2src = n2s[:, :gw]
            emit_rsqrt(
                nc, rng[:, :gw], n2src, nsa[:, :gw], nsb[:, :gw], gw,
                fold=1.0 / CS,
            )
            # 4) per XB batch: PE transposes into one PSUM strip, ONE u16
            #    copyback, DoubleRow matmuls, Exp, den, normalize, store
            for tm in range(t0 // XB, (t1 + XB - 1) // XB):
                bt0 = max(t0, tm * XB)
                bt1 = min(t1, (tm + 1) * XB)
                bw = bt1 - bt0
                xm = xmacs[tm]
                xt = xt_pool.tile([P, XB * D], xdt, tag="xt")
                # PSUM strips of SB tiles (2 banks each) -> one copyback per
                # strip through a uint16 view (keeps the DVE 2x copy mode)
                SB = XB if XDT == "fp8" else 2
                for ss in range(bt0, bt1, SB):
                    se = min(ss + SB, bt1)
                    tps = tps_pool.tile([P, SB * D], xdt, tag="tps")
                    for t in range(ss, se):
                        xs = xm[:, D * (t % XB) : D * (t % XB + 1)]
                        tbase = D * (t - ss)
                        if XDT == "fp8":
                            # chunk pairs interleaved at step 2 via the
                            # flipped identity:
                            # xt[p, 256c+2j+i] = x[127-j, 256c+128i+p]
                            for cpr in range(ND // 2):
                                base = tbase + 2 * P * cpr
                                reg = tps[:, base : base + 2 * P].rearrange(
                                    "p (n two) -> p two n", two=2
                                )
                                for i in range(2):
                                    nc.tensor.transpose(
                                        reg[:, i],
                                        xs[
                                            :,
                                            P * (2 * cpr + i) : P
                                            * (2 * cpr + i + 1),
                                        ],
                                        identa[:],
                                    )
                        else:
                            for b in range(ND):
                                nc.tensor.transpose(
                                    tps[:, tbase + P * b : tbase + P * (b + 1)],
                                    xs[:, P * b : P * (b + 1)],
                                    ident[:],
                                )
                    sw = (se - ss) * u16_per_tile
                    xtu = xt[:].bitcast(dt.uint16)[
                        :,
                        (ss - bt0) * u16_per_tile : (ss - bt0) * u16_per_tile
                        + sw,
                    ]
                    tpu = tps[:].bitcast(dt.uint16)[:, :sw]
                    if CB_MODE == "split" and 0 < CB_F1 < sw:
                        nc.vector.tensor_copy(xtu[:, :CB_F1], tpu[:, :CB_F1])
                        nc.scalar.copy(xtu[:, CB_F1:], tpu[:, CB_F1:])
                    else:
                        nc.vector.tensor_copy(xtu, tpu)
                for t in range(bt0, bt1):
                    xtt = xt[:, D * (t % XB) : D * (t % XB + 1)]
                    sps = sps_pool.tile([P, K], dt.float32, tag="sps")
                    if XDT == "fp8":
                        for b in range(ND // 2):
                            # weights: flat pre-interleaved (SwInterleave);
                            # moving: k outer (step 2), pair i inner (step 1)
                            lhsT = xtt[:, 2 * P * b : 2 * P * (b + 1)]
                            rhs = cnT[
                                :, 2 * K * b : 2 * K * (b + 1)
                            ].rearrange("p (k two) -> p two k", two=2)
                            nc.tensor.matmul(
                                sps[:], lhsT, rhs,
                                start=(b == 0), stop=(b == ND // 2 - 1),
                                perf_mode=(
                                    mybir.MatmulPerfMode.DoubleRowSwInterleave
                                ),
                            )
                    else:
                        for b in range(ND):
                            nc.tensor.matmul(
                                sps[:],
                                xtt[:, P * b : P * (b + 1)],
                                cnT[:, K * b : K * (b + 1)],
                                start=(b == 0), stop=(b == ND - 1),
                            )
                    if t == bt0:
                        e = e_pool.tile([P, XB * K], dt.float16, tag="e")
                    j = t - t0
                    if DEN_MODE == "act":
                        nc.scalar.activation(
                            e[:, K * (t % XB) : K * (t % XB + 1)], sps[:],
                            AFT.Exp, scale=rng[:, j : j + 1],
                            accum_out=denscols[:, t : t + 1],
                        )
                    else:
                        nc.scalar.activation(
                            e[:, K * (t % XB) : K * (t % XB + 1)], sps[:],
                            AFT.Exp, scale=rng[:, j : j + 1],
                        )
                if DEN_MODE == "dve":
                    den = den_pool.tile([P, XB], dt.float32, tag="den")
                    nc.vector.tensor_reduce(
                        den[:, :bw],
                        e[:, : K * bw].rearrange("p (s k) -> p s k", s=bw),
                        mybir.AxisListType.X,
                        Alu.add,
                    )
                    densrc = den[:, :bw]
                    rden = den_pool.tile([P, XB], dt.float32, tag="rden")
                    rdst = rden[:, :bw]
                else:
                    densrc = denscols[:, bt0:bt1]
                    rdst = rdenscols[:, bt0:bt1]
                nc.vector.reciprocal(rdst, densrc)
                pm = pm_pool.tile([P, XB * K], dt.float16, tag="pm")
                for t in range(bt0, bt1):
                    rcol = rdst[:, t - bt0 : t - bt0 + 1]
                    if MUL_MODE == "dve":
                        nc.vector.tensor_scalar_mul(
                            pm[:, K * (t % XB) : K * (t % XB + 1)],
                            e[:, K * (t % XB) : K * (t % XB + 1)],
                            rcol,
                        )
                    else:
                        nc.scalar.activation(
                            pm[:, K * (t % XB) : K * (t % XB + 1)],
                            e[:, K * (t % XB) : K * (t % XB + 1)],
                            AFT.Copy, scale=rcol,
                        )
                dst = o_d.ap()[
                    P * XB * tm : P * XB * (tm + 1), :
                ].rearrange("(s p) k -> p s k", s=XB)
                nc.sync.dma_start(
                    dst, pm[:].rearrange("p (s k) -> p s k", s=XB)
                )

    nc.compile()
    return nc


_cache = {}


def _get_nc(npc=NPC):
    if npc not in _cache:
        _cache[npc] = build_bass(npc)
    return _cache[npc]


def kernel(batch: np.ndarray, centroids: np.ndarray) -> np.ndarray:
    from concourse.bass_utils import run_bass_kernel_spmd

    assert batch.shape == (N, D) and centroids.shape == (K, D)
    batch = np.ascontiguousarray(batch, dtype=np.float32)
    centroids = np.ascontiguousarray(centroids, dtype=np.float32)

    nc = _get_nc()
    in_maps = [
        {"x": batch[i * NPC : (i + 1) * NPC], "c": centroids}
        for i in range(NCORES)
    ]
    res = run_bass_kernel_spmd(nc, in_maps, core_ids=list(range(NCORES)))
    out = np.concatenate(
        [np.asarray(res.results[i]["o"]) for i in range(NCORES)], axis=0
    )
    return out.astype(np.float32)


# revision 14
# speedup vs baseline: 1.7441x; 1.0737x over previous
"""VQ codebook cosine-similarity softmax kernel for Trainium2 (8 NeuronCores).

Computes softmax(cos_sim(batch, centroids)) for batch [131072, 1024] f32 and
centroids [256, 1024] f32, data-parallel over the batch dim across 8 cores.

Per-core pipeline (16384 rows), v3 (fp8 DoubleRow + batched u16 copyback):
  - SWDGE cast-DMA loads x tiles HBM f32 -> SBUF fp8e4 (halves SBUF traffic,
    feeds double-pumped fp8 matmuls; rel err ~7e-3 vs the 2e-2 budget)
  - row norms^2 on the fp8 tiles, split column-wise between DVE
    (scalar_tensor_tensor) and ACT (Square), both with accum_out
  - batched rsqrt (0x5f3759df bit trick + 3 Newton steps on DVE); the last
    Newton step folds in 1/CS (centroid prescale) for the Exp scale
  - PE transposes each [128,128] fp8 block into a [128, XB*1024] PSUM strip;
    ONE tensor_copy per XB batch moves it to SBUF through a uint16 bitcast
    (2-byte dtype unlocks the DVE 2x mode; fp8 would run 1x)
  - 4 DoubleRow fp8 matmuls per row tile (contraction 256 each): weights
    xt8[:, (two n)] pair stride 128B, moving cnT[:, (two k)] pair stride
    256B - the block layout dual-fp8 LDWEIGHTS requires (stride%16==0)
  - softmax: logits = cos in [-1,1] so no max-subtraction needed; ACT
    Exp(scale=1/(||x||*CS)) with accum_out denominator; batched DVE
    reciprocal per XB; DVE tensor_scalar_mul (4x mode) normalize -> fp16
  - fp16 output store (host upcasts to f32)
"""

import os
import sys

if "/opt/trn_rl_repo" not in sys.path:
    sys.path.insert(0, "/opt/trn_rl_repo")

import numpy as np

# fp8 is dead on TRN2 HW: fp8 PE transposes must write element-step-2 pairs
# (odd-offset PSUM writes are illegal), and dual-fp8 LDWEIGHTS wants 16B-
# aligned pair strides no cheap producer emits. fp16 is the working path.
XDT = os.environ.get("KM_XDT", "fp16")  # fp8 | fp16
# columns of each row-tile squared+accumulated on DVE (sts); rest on ACT
SQ_SPLIT = int(os.environ.get("KM_SQ_SPLIT", "512"))
DEN_MODE = os.environ.get("KM_DEN_MODE", "act")  # act (Exp accum) | dve (reduce)
NEWTON = int(os.environ.get("KM_NEWTON", "2"))  # rsqrt Newton steps
MUL_MODE = os.environ.get("KM_MUL_MODE", "dve")  # dve (ts_mul) | act (Copy scale)
CB_MODE = os.environ.get("KM_CB_MODE", "dve")  # copyback: dve | split
CB_F1 = int(os.environ.get("KM_CB_F1", "3072"))  # split: u16 cols on DVE
X_BUFS = int(os.environ.get("KM_X_BUFS", "8"))
XT_BUFS = int(os.environ.get("KM_XT_BUFS", "3"))
E_BUFS = int(os.environ.get("KM_E_BUFS", "4"))
PM_BUFS = int(os.environ.get("KM_PM_BUFS", "3"))
SPS_BUFS = int(os.environ.get("KM_SPS_BUFS", "4"))
TPS_BUFS = int(os.environ.get("KM_TPS_BUFS", "2"))

N, D, K = 131072, 1024, 256
NCORES = 8
NPC = N // NCORES  # rows per core
P = 128  # partitions / tile rows
XB = 4  # row-tiles per load/store DMA batch and copyback strip
G = 16  # row-tiles per norm group (batched rsqrt)
CS = 32.0  # centroid prescale: keeps cn out of fp8e4 subnormals

RSQRT_MAGIC = 0x5F3759DF


def build_bass(npc=NPC):
    from contextlib import ExitStack

    import concourse.bacc as bacc
    import concourse.mybir as mybir
    import concourse.tile as tile
    from concourse.masks import make_identity

    dt = mybir.dt
    AFT = mybir.ActivationFunctionType
    Alu = mybir.AluOpType
    xdt = dt.float8e4 if XDT == "fp8" else dt.float16

    nt = npc // P  # row tiles
    assert npc % (P * XB) == 0
    ngroups = (nt + G - 1) // G
    ND = D // P  # 128-wide d-chunks (8)

    nc = bacc.Bacc(
        "TRN2", target_bir_lowering=False, debug=False, num_devices=NCORES
    )
    x_d = nc.dram_tensor("x", [npc, D], dt.float32, kind="ExternalInput")
    c_d = nc.dram_tensor("c", [K, D], dt.float32, kind="ExternalInput")
    o_d = nc.dram_tensor("o", [npc, K], dt.float16, kind="ExternalOutput")

    def emit_rsqrt(nc, dst, src, scratch_a, scratch_b, w, fold=1.0):
        """dst[:, :w] = fold/sqrt(src[:, :w]) via bit trick + 3 Newton steps."""
        srci = src.bitcast(dt.int32)
        dsti = dst.bitcast(dt.int32)
        nc.vector.tensor_scalar(dsti, srci, 1, None, Alu.logical_shift_right)
        nc.vector.tensor_scalar(dsti, dsti, -1, None, Alu.bitwise_xor)
        nc.vector.tensor_scalar(dsti, dsti, RSQRT_MAGIC + 1, None, Alu.add)
        for it in range(NEWTON):
            nc.vector.tensor_tensor(scratch_a, dst, dst, Alu.mult)
            nc.vector.tensor_tensor(scratch_b, scratch_a, src, Alu.mult)
            nc.vector.tensor_scalar(
                scratch_b, scratch_b, -0.5, 1.5, Alu.mult, Alu.add
            )
            if it == NEWTON - 1 and fold != 1.0:
                nc.vector.scalar_tensor_tensor(
                    dst, dst, fold, scratch_b, Alu.mult, Alu.mult
                )
            else:
                nc.vector.tensor_tensor(dst, dst, scratch_b, Alu.mult)

    def make_antidiag(nc, ap):
        """Antidiagonal permutation: ap[x, y] = 1.0 iff x + y == P-1."""
        nc.gpsimd.memset(ap, 0.0)
        nc.gpsimd.affine_select(
            out=ap,
            in_=ap,
            compare_op=Alu.not_equal,
            fill=1.0,
            base=-(P - 1),
            pattern=[[1, P]],
            channel_multiplier=1,
        )

    with tile.TileContext(nc) as tc, ExitStack() as ctx:
        const = ctx.enter_context(tc.tile_pool(name="const", bufs=1))
        ident = const.tile([P, P], xdt)
        make_identity(nc, ident[:])
        if XDT == "fp8":
            # fp8 PE transposes must write element-step-2 (pair-interleaved)
            # outputs; x transposes use a flipped identity so the column
            # reversal of DoubleRowSwInterleave weight reads cancels out.
            identa = const.tile([P, P], xdt)
            make_antidiag(nc, identa[:])

        # cnT: [128 d-in-chunk, ND*K] with chunk b at cols [K*b, K*b+K)
        cnT = const.tile([P, ND * K], xdt)
        n2a = const.tile([P, max(nt, 1)], dt.float32)
        n2b = const.tile([P, max(nt, 1)], dt.float32)
        denscols = const.tile([P, max(nt, 1)], dt.float32)
        rdenscols = const.tile([P, max(nt, 1)], dt.float32)

        # ---- centroid prep (one-time) ----
        with ExitStack() as _cstack:
            cprep = _cstack.enter_context(tc.tile_pool(name="cprep", bufs=2))
            cpsum = _cstack.enter_context(
                tc.tile_pool(name="cpsum", bufs=2, space="PSUM")
            )
            for h in range(K // P):  # 2 halves of the K=256 centroids
                c32 = cprep.tile([P, D], dt.float32, tag="c32")
                nc.sync.dma_start(c32[:], c_d.ap()[P * h : P * (h + 1), :])
                csq = cprep.tile([P, D], dt.float32, tag="csq")
                cn2 = cprep.tile([P, 1], dt.float32, tag="cn2")
                nc.vector.scalar_tensor_tensor(
                    csq[:], c32[:], 1.0, c32[:], Alu.mult, Alu.mult,
                    accum_out=cn2[:],
                )
                crn = cprep.tile([P, 1], dt.float32, tag="crn")
                csa = cprep.tile([P, 1], dt.float32, tag="csa")
                csb = cprep.tile([P, 1], dt.float32, tag="csb")
                emit_rsqrt(nc, crn[:], cn2[:], csa[:], csb[:], 1)
                # cnrows = c * (CS/||c||) in the matmul dtype
                cnrows = cprep.tile([P, D], xdt, tag="cnrows")
                nc.vector.tensor_scalar(
                    cnrows[:], c32[:], crn[:], CS, Alu.mult, Alu.mult
                )
                if XDT == "fp8":
                    # chunk pairs (2c, 2c+1) interleaved at element step 2:
                    # cnT[p, 512c + 2k + i] = cn[k-half h, d=128(2c+i)+p]
                    for cpr in range(ND // 2):
                        pt = cpsum.tile([P, 2 * P], xdt, tag="ct_ps")
                        pt2 = pt[:].rearrange("p (n two) -> p two n", two=2)
                        for i in range(2):
                            nc.tensor.transpose(
                                pt2[:, i],
                                cnrows[
                                    :, P * (2 * cpr + i) : P * (2 * cpr + i + 1)
                                ],
                                ident[:],
                            )
                        nc.vector.tensor_copy(
                            cnT[:, 2 * K * cpr + 2 * P * h :
                                2 * K * cpr + 2 * P * (h + 1)].bitcast(
                                dt.uint16
                            ),
                            pt[:].bitcast(dt.uint16),
                        )
                else:
                    for b in range(ND):
                        pt = cpsum.tile([P, P], xdt, tag="ct_ps")
                        nc.tensor.transpose(
                            pt[:], cnrows[:, P * b : P * (b + 1)], ident[:]
                        )
                        nc.vector.tensor_copy(
                            cnT[:, K * b + P * h : K * b + P * h + P], pt[:]
                        )

        # ---- main loop ----
        x_pool = ctx.enter_context(tc.tile_pool(name="xm", bufs=X_BUFS))
        xt_pool = ctx.enter_context(tc.tile_pool(name="xt", bufs=XT_BUFS))
        sq_pool = ctx.enter_context(tc.tile_pool(name="sq", bufs=2))
        e_pool = ctx.enter_context(tc.tile_pool(name="e", bufs=E_BUFS))
        pm_pool = ctx.enter_context(tc.tile_pool(name="pm", bufs=PM_BUFS))
        nrm_pool = ctx.enter_context(tc.tile_pool(name="nrm", bufs=4))
        den_pool = ctx.enter_context(tc.tile_pool(name="den", bufs=4))
        tps_pool = ctx.enter_context(
            tc.tile_pool(name="tps", bufs=TPS_BUFS, space="PSUM")
        )
        sps_pool = ctx.enter_context(
            tc.tile_pool(name="sps", bufs=SPS_BUFS, space="PSUM")
        )

        sd = max(0, min(D, SQ_SPLIT))
        u16_per_tile = D // 2 if XDT == "fp8" else D
        for g in range(ngroups):
            t0 = g * G
            t1 = min(t0 + G, nt)
            gtiles = range(t0, t1)
            gw = t1 - t0
            # 1) cast-loads (XB row-tiles per DMA)
            xmacs = {}
            for tm in range(t0 // XB, (t1 + XB - 1) // XB):
                xm = x_pool.tile([P, XB * D], xdt, tag="xm")
                src = x_d.ap()[P * XB * tm : P * XB * (tm + 1), :].rearrange(
                    "(s p) d -> p s d", s=XB
                )
                nc.gpsimd.dma_start(
                    xm[:].rearrange("p (s d) -> p s d", s=XB), src
                )
                xmacs[tm] = xm
            # 2) row norms^2, split column-wise across DVE (sts) and ACT (Sq)
            for t in gtiles:
                xm = xmacs[t // XB]
                xs = xm[:, D * (t % XB) : D * (t % XB + 1)]
                if sd > 0:
                    sqa = sq_pool.tile([P, D], xdt, tag="sqa")
                    nc.vector.scalar_tensor_tensor(
                        sqa[:, :sd], xs[:, :sd], 1.0, xs[:, :sd],
                        Alu.mult, Alu.mult, accum_out=n2a[:, t : t + 1],
                    )
                if sd < D:
                    sqb = sq_pool.tile([P, D], xdt, tag="sqb")
                    nc.scalar.activation(
                        sqb[:, sd:], xs[:, sd:], AFT.Square,
                        accum_out=n2b[:, t : t + 1],
                    )
            # 3) batched rsqrt; folds 1/CS into the Exp scale
            rng = nrm_pool.tile([P, G], dt.float32, tag="rng")
            nsa = nrm_pool.tile([P, G], dt.float32, tag="nsa")
            nsb = nrm_pool.tile([P, G], dt.float32, tag="nsb")
            n2s = nrm_pool.tile([P, G], dt.float32, tag="n2s")
            if sd == 0:
                n2src = n2b[:, t0:t1]
            elif sd == D:
                n2src = n2a[:, t0:t1]
            else:
                nc.vector.tensor_tensor(
                    n2s[:, :gw], n2a[:, t0:t1], n2b[:, t0:t1], Alu.add
                )
                n2src = n2s[:, :gw]
            emit_rsqrt(
                nc, rng[:, :gw], n2src, nsa[:, :gw], nsb[:, :gw], gw,
                fold=1.0 / CS,
            )
            # 4) per XB batch: PE transposes into one PSUM strip, ONE u16
            #    copyback, DoubleRow matmuls, Exp, den, normalize, store
            for tm in range(t0 // XB, (t1 + XB - 1) // XB):
                bt0 = max(t0, tm * XB)
                bt1 = min(t1, (tm + 1) * XB)
                bw = bt1 - bt0
                xm = xmacs[tm]
                xt = xt_pool.tile([P, XB * D], xdt, tag="xt")
                # PSUM strips of SB tiles (2 banks each) -> one copyback per
                # strip through a uint16 view (keeps the DVE 2x copy mode)
                SB = XB if XDT == "fp8" else 2
                for ss in range(bt0, bt1, SB):
                    se = min(ss + SB, bt1)
                    tps = tps_pool.tile([P, SB * D], xdt, tag="tps")
                    for t in range(ss, se):
                        xs = xm[:, D * (t % XB) : D * (t % XB + 1)]
                        tbase = D * (t - ss)
                        if XDT == "fp8":
                            # chunk pairs interleaved at step 2 via the
                            # flipped identity:
                            # xt[p, 256c+2j+i] = x[127-j, 256c+128i+p]
                            for cpr in range(ND // 2):
                                base = tbase + 2 * P * cpr
                                reg = tps[:, base : base + 2 * P].rearrange(
                                    "p (n two) -> p two n", two=2
                                )
                                for i in range(2):
                                    nc.tensor.transpose(
                                        reg[:, i],
                                        xs[
                                            :,
                                            P * (2 * cpr + i) : P
                                            * (2 * cpr + i + 1),
                                        ],
                                        identa[:],
                                    )
                        else:
                            for b in range(ND):
                                nc.tensor.transpose(
                                    tps[:, tbase + P * b : tbase + P * (b + 1)],
                                    xs[:, P * b : P * (b + 1)],
                                    ident[:],
                                )
                    sw = (se - ss) * u16_per_tile
                    xtu = xt[:].bitcast(dt.uint16)[
                        :,
                        (ss - bt0) * u16_per_tile : (ss - bt0) * u16_per_tile
                        + sw,
                    ]
                    tpu = tps[:].bitcast(dt.uint16)[:, :sw]
                    if CB_MODE == "split" and 0 < CB_F1 < sw:
                        nc.vector.tensor_copy(xtu[:, :CB_F1], tpu[:, :CB_F1])
                        nc.scalar.copy(xtu[:, CB_F1:], tpu[:, CB_F1:])
                    else:
                        nc.vector.tensor_copy(xtu, tpu)
                for t in range(bt0, bt1):
                    xtt = xt[:, D * (t % XB) : D * (t % XB + 1)]
                    sps = sps_pool.tile([P, K], dt.float32, tag="sps")
                    if XDT == "fp8":
                        for b in range(ND // 2):
                            # weights: flat pre-interleaved (SwInterleave);
                            # moving: k outer (step 2), pair i inner (step 1)
                            lhsT = xtt[:, 2 * P * b : 2 * P * (b + 1)]
                            rhs = cnT[
                                :, 2 * K * b : 2 * K * (b + 1)
                            ].rearrange("p (k two) -> p two k", two=2)
                            nc.tensor.matmul(
                                sps[:], lhsT, rhs,
                                start=(b == 0), stop=(b == ND // 2 - 1),
                                perf_mode=(
                                    mybir.MatmulPerfMode.DoubleRowSwInterleave
                                ),
                            )
                    else:
                        for b in range(ND):
                            nc.tensor.matmul(
                                sps[:],
                                xtt[:, P * b : P * (b + 1)],
                                cnT[:, K * b : K * (b + 1)],
                                start=(b == 0), stop=(b == ND - 1),
                            )
                    if t == bt0:
                        e = e_pool.tile([P, XB * K], dt.float16, tag="e")
                    j = t - t0
                    if DEN_MODE == "act":
                        nc.scalar.activation(
                            e[:, K * (t % XB) : K * (t % XB + 1)], sps[:],
                            AFT.Exp, scale=rng[:, j : j + 1],
                            accum_out=denscols[:, t : t + 1],
                        )
                    else:
                        nc.scalar.activation(
                            e[:, K * (t % XB) : K * (t % XB + 1)], sps[:],
                            AFT.Exp, scale=rng[:, j : j + 1],
                        )
                if DEN_MODE == "dve":
                    den = den_pool.tile([P, XB], dt.float32, tag="den")
                    nc.vector.tensor_reduce(
                        den[:, :bw],
                        e[:, : K * bw].rearrange("p (s k) -> p s k", s=bw),
                        mybir.AxisListType.X,
                        Alu.add,
                    )
                    densrc = den[:, :bw]
                    rden = den_pool.tile([P, XB], dt.float32, tag="rden")
                    rdst = rden[:, :bw]
                else:
                    densrc = denscols[:, bt0:bt1]
                    rdst = rdenscols[:, bt0:bt1]
                nc.vector.reciprocal(rdst, densrc)
                pm = pm_pool.tile([P, XB * K], dt.float16, tag="pm")
                for t in range(bt0, bt1):
                    rcol = rdst[:, t - bt0 : t - bt0 + 1]
                    if MUL_MODE == "dve":
                        nc.vector.tensor_scalar_mul(
                            pm[:, K * (t % XB) : K * (t % XB + 1)],
                            e[:, K * (t % XB) : K * (t % XB + 1)],
                            rcol,
                        )
                    else:
                        nc.scalar.activation(
                            pm[:, K * (t % XB) : K * (t % XB + 1)],
                            e[:, K * (t % XB) : K * (t % XB + 1)],
                            AFT.Copy, scale=rcol,
                        )
                dst = o_d.ap()[
                    P * XB * tm : P * XB * (tm + 1), :
                ].rearrange("(s p) k -> p s k", s=XB)
                nc.sync.dma_start(
                    dst, pm[:].rearrange("p (s k) -> p s k", s=XB)
                )

    nc.compile()
    return nc


_cache = {}


def _get_nc(npc=NPC):
    if npc not in _cache:
        _cache[npc] = build_bass(npc)
    return _cache[npc]


def kernel(batch: np.ndarray, centroids: np.ndarray) -> np.ndarray:
    from concourse.bass_utils import run_bass_kernel_spmd

    assert batch.shape == (N, D) and centroids.shape == (K, D)
    batch = np.ascontiguousarray(batch, dtype=np.float32)
    centroids = np.ascontiguousarray(centroids, dtype=np.float32)

    nc = _get_nc()
    in_maps = [
        {"x": batch[i * NPC : (i + 1) * NPC], "c": centroids}
        for i in range(NCORES)
    ]
    res = run_bass_kernel_spmd(nc, in_maps, core_ids=list(range(NCORES)))
    out = np.concatenate(
        [np.asarray(res.results[i]["o"]) for i in range(NCORES)], axis=0
    )
    return out.astype(np.float32)
